# revision 18
# baseline (speedup 1.0000x reference)
"""Self-contained TRN2 Bass kernel for nn_CAM_Module (channel attention).

kernel(x, gamma): x [16,512,64,64] f32, gamma [1] f32 -> [16,512,64,64] f32.
Data-parallel over batch: 2 samples per NeuronCore across 8 cores.

Math: q = x.reshape(B,C,HW); E = q@q.T; softmax(rowmax(E)-E) == softmax(-E)
(shift invariance), computed as exp(rowmin(E)-E)/rowsum; out = gamma*(A@q)+x.

On-chip strategy (per core, 2 samples):
  - load fp32, cast to fp16 (DVE/ACT), PE-transpose 128x128 tiles to build
    q^T chunks; single-pass fp16 Gram accumulated in fp32 PSUM, computing
    only upper-triangle blocks (E symmetric), mirrored via PE transposes.
  - softmax via ACT exp(scale=-1, bias=rowmin) with fused rowsum accum_out.
  - A-matmul fp16: lhsT = transpose(exp), rhs = fp16(q); epilogue fused in
    one DVE scalar_tensor_tensor: out = psum*(gamma/rowsum) + fp16(x),
    written as fp16 (output tensor stored f16 to halve output HBM traffic;
    host converts back to f32 -- costs ~2e-4 relative quantization noise).
"""
import sys
if '/opt/trn_rl_repo' not in sys.path:
    sys.path.insert(0, '/opt/trn_rl_repo')
import numpy as np
import concourse.bass as bass
import concourse.tile as tile
import concourse.mybir as mybir
from concourse.masks import make_identity

F32 = mybir.dt.float32
F16 = mybir.dt.float16

C = 512          # channels
N = 4096         # spatial (64*64)
CB = C // 128    # 4 c-blocks
NK = N // 128    # 32 transpose chunks
NG = NK // 2     # 16 transpose groups (2 chunks per PSUM bounce bank)
NO = N // 512    # 8 output column chunks
NP = 8          # load/cast pieces per row-block (512 cols each)
PW = N // NP     # piece width
S = 2            # samples per core
PRE = 8          # sample-1 transpose groups hoisted before sample-0 softmax


def build(nc: bass.Bass):
    x_ext = nc.declare_dram_parameter("x", [S * C, N], F32, isOutput=False)
    g_ext = nc.declare_dram_parameter("gamma", [1, 1], F32, isOutput=False)
    out_ext = nc.declare_dram_parameter("out", [S * C, N], F16, isOutput=True)
    x_ap = x_ext.ap()
    out_ap = out_ext.ap()
    EW = [512 - 128 * m for m in range(CB)]

    with tile.TileContext(nc) as tc:
        with (
            tc.tile_pool(name="const", bufs=1) as const,
            tc.tile_pool(name="x32", bufs=12) as x32,
            tc.tile_pool(name="q16", bufs=2 * CB * NP) as q16p,
            tc.tile_pool(name="qt", bufs=14) as qtp,
            tc.tile_pool(name="esb", bufs=2) as esbp,
            tc.tile_pool(name="expn", bufs=2) as expnp,
            tc.tile_pool(name="expt", bufs=2 * CB) as exptp,
            tc.tile_pool(name="vecs", bufs=4 * CB) as vecs,
            tc.tile_pool(name="outs", bufs=7) as outsp,
            tc.tile_pool(name="ps_bounce", bufs=2, space="PSUM") as ps_t,
            tc.tile_pool(name="ps_e", bufs=1, space="PSUM") as ps_e,
            tc.tile_pool(name="ps_o", bufs=2, space="PSUM") as ps_o,
        ):
            ident = const.tile([128, 128], F16)
            make_identity(nc, ident)
            ident32 = const.tile([128, 128], F32)
            make_identity(nc, ident32)
            gbc = const.tile([128, 1], F32)
            nc.gpsimd.dma_start(out=gbc, in_=g_ext.ap().to_broadcast((128, 1)))

            st = [dict() for _ in range(S)]

            def load(s):
                # 512KB pieces so compute starts early; q16[cb][p] covers
                # columns [p*PW, (p+1)*PW)
                q16 = [[None] * NP for _ in range(CB)]
                for p in range(NP):
                    for cb in range(CB):
                        qc = q16p.tile([128, PW], F16, tag="q16",
                                       name=f"q16_{s}_{cb}_{p}")
                        if p <= 1:
                            # first pieces in 256-col halves so the early
                            # transposes start sooner
                            for hh in range(2):
                                xt = x32.tile([128, PW // 2], F32, tag="xth",
                                              bufs=4,
                                              name=f"xt_{s}_{cb}_{p}_{hh}")
                                nc.sync.dma_start(
                                    out=xt,
                                    in_=x_ap[
                                        s * C + cb * 128
                                        : s * C + (cb + 1) * 128,
                                        p * PW + hh * (PW // 2)
                                        : p * PW + (hh + 1) * (PW // 2),
                                    ],
                                )
                                dst = qc[:, hh * (PW // 2) :
                                         (hh + 1) * (PW // 2)]
                                if (cb + hh) % 2 == 0:
                                    nc.vector.tensor_copy(dst, xt[:])
                                else:
                                    nc.scalar.copy(dst, xt[:])
                            q16[cb][p] = qc
                            continue
                        xt = x32.tile([128, PW], F32, tag="xt",
                                      name=f"xt_{s}_{cb}_{p}")
                        nc.sync.dma_start(
                            out=xt,
                            in_=x_ap[
                                s * C + cb * 128 : s * C + (cb + 1) * 128,
                                p * PW : (p + 1) * PW,
                            ],
                        )
                        if (p * CB + cb) % 2 == 0:
                            nc.vector.tensor_copy(qc[:], xt[:])
                        else:
                            nc.scalar.copy(qc[:], xt[:])
                        q16[cb][p] = qc
                st[s]["q16"] = q16
                st[s]["qtc"] = {}

            def tgroup(s, g):
                # transpose 2 chunks (8 [128,128] fp16 tiles) into one
                # PSUM bounce bank, evacuate to SBUF in one op
                q16 = st[s]["q16"]
                # odd groups bounce through the (idle-during-Gram) ps_o
                # pool: an effective 4-deep transpose ring, so T0 of
                # group g waits evac(g-4) instead of evac(g-2)
                pool, tag = (ps_t, "bounce") if g % 2 == 0 else (ps_o, "acc")
                bounce = pool.tile([128, 2, CB, 128], F16, tag=tag,
                                   name=f"bounce_{s}_{g}")
                for h in range(2):
                    k = 2 * g + h
                    kp, ko = k // (PW // 128), (k % (PW // 128)) * 128
                    for cb in range(CB):
                        nc.tensor.transpose(
                            bounce[:, h, cb, :],
                            q16[cb][kp][:, ko : ko + 128],
                            ident,
                        )
                qtc = qtp.tile([128, 2, CB * 128], F16, tag="qtc",
                               name=f"qtc_{s}_{g}")
                if g % 2 == 0:
                    nc.scalar.copy(qtc[:], bounce[:, :, :, :])
                else:
                    nc.vector.tensor_copy(qtc[:], bounce[:, :, :, :])
                st[s]["qtc"][g] = qtc

            def emm(s, g):
                # symmetric Gram accumulation: upper-triangle blocks only
                if "E" not in st[s]:
                    st[s]["E"] = ps_e.tile([128, CB, 512], F32, tag="E",
                                           name=f"E_{s}")
                E = st[s]["E"]
                qtc = st[s]["qtc"][g]
                for h in range(2):
                    k = 2 * g + h
                    for m in range(CB):
                        nc.tensor.matmul(
                            E[:, m, 0 : EW[m]],
                            lhsT=qtc[:, h, m * 128 : (m + 1) * 128],
                            rhs=qtc[:, h, m * 128 : 512],
                            start=(k == 0),
                            stop=(k == NK - 1),
                        )

            def softmax(s):
                # rebuild full E rows in SBUF (mirror lower triangle),
                # then exp(rowmin - E) with fused rowsum
                E = st[s]["E"]
                E_sb = esbp.tile([128, CB, 512], F32, tag="esb",
                                 name=f"esb_{s}")
                for m in range(CB):
                    if m % 2 == 0:
                        nc.scalar.copy(E_sb[:, m, m * 128 : 512],
                                       E[:, m, 0 : EW[m]])
                    else:
                        nc.vector.tensor_copy(E_sb[:, m, m * 128 : 512],
                                              E[:, m, 0 : EW[m]])
                for i in range(CB):
                    for j in range(i):
                        tb = ps_o.tile([128, 128], F32, tag="acc",
                                       name=f"tb_{s}_{i}_{j}")
                        nc.tensor.transpose(
                            tb[:], E_sb[:, j, i * 128 : (i + 1) * 128], ident32
                        )
                        if (i + j) % 2 == 0:
                            nc.scalar.copy(
                                E_sb[:, i, j * 128 : (j + 1) * 128], tb[:])
                        else:
                            nc.vector.tensor_copy(
                                E_sb[:, i, j * 128 : (j + 1) * 128], tb[:])
                expn = expnp.tile([128, CB, 512], F16, tag="expn",
                                  name=f"expn_{s}")
                scales = []
                for m in range(CB):
                    mv = vecs.tile([128, 1], F32, tag="mv", name=f"mv_{s}_{m}")
                    nc.vector.tensor_reduce(
                        mv, E_sb[:, m, :], axis=mybir.AxisListType.X,
                        op=mybir.AluOpType.min,
                    )
                    Z = vecs.tile([128, 1], F32, tag="Z", name=f"Z_{s}_{m}")
                    nc.scalar.activation(
                        expn[:, m, :],
                        E_sb[:, m, :],
                        mybir.ActivationFunctionType.Exp,
                        bias=mv,
                        scale=-1.0,
                        accum_out=Z,
                    )
                    rz = vecs.tile([128, 1], F32, tag="rz", name=f"rz_{s}_{m}")
                    nc.vector.reciprocal(rz, Z)
                    sc = vecs.tile([128, 1], F32, tag="sc", name=f"sc_{s}_{m}")
                    nc.vector.tensor_mul(sc, rz, gbc)  # gamma / Z
                    scales.append(sc)
                st[s]["expn"] = expn
                st[s]["scales"] = scales

            def softmax_esb_mirror(s):
                # E_sb rebuild with copies on ACT (DVE is busy with the
                # a0 epilogue STTs this runs under)
                E = st[s]["E"]
                E_sb = esbp.tile([128, CB, 512], F32, tag="esb",
                                 name=f"esb_{s}")
                for m in range(CB):
                    nc.scalar.copy(E_sb[:, m, m * 128 : 512],
                                   E[:, m, 0 : EW[m]])
                for i in range(CB):
                    for j in range(i):
                        tb = ps_o.tile([128, 128], F32, tag="acc",
                                       name=f"tb_{s}_{i}_{j}")
                        nc.tensor.transpose(
                            tb[:], E_sb[:, j, i * 128 : (i + 1) * 128], ident32
                        )
                        nc.scalar.copy(
                            E_sb[:, i, j * 128 : (j + 1) * 128], tb[:])
                st[s]["E_sb"] = E_sb

            def softmax_rowmin(s, ms):
                E_sb = st[s]["E_sb"]
                mvs = st[s].setdefault("mvs", {})
                for m in ms:
                    mv = vecs.tile([128, 1], F32, tag="mv", name=f"mv_{s}_{m}")
                    nc.vector.tensor_reduce(
                        mv, E_sb[:, m, :], axis=mybir.AxisListType.X,
                        op=mybir.AluOpType.min,
                    )
                    mvs[m] = mv

            def softmax_exp(s, ms):
                E_sb = st[s]["E_sb"]
                expn = st[s].setdefault(
                    "expn",
                    expnp.tile([128, CB, 512], F16, tag="expn",
                               name=f"expn_{s}"))
                Zs = st[s].setdefault("Zs", {})
                for m in ms:
                    Z = vecs.tile([128, 1], F32, tag="Z", name=f"Z_{s}_{m}")
                    nc.scalar.activation(
                        expn[:, m, :],
                        E_sb[:, m, :],
                        mybir.ActivationFunctionType.Exp,
                        bias=st[s]["mvs"][m],
                        scale=-1.0,
                        accum_out=Z,
                    )
                    Zs[m] = Z

            def softmax_scales(s):
                scales = []
                for m in range(CB):
                    rz = vecs.tile([128, 1], F32, tag="rz", name=f"rz_{s}_{m}")
                    nc.vector.reciprocal(rz, st[s]["Zs"][m])
                    sc = vecs.tile([128, 1], F32, tag="sc", name=f"sc_{s}_{m}")
                    nc.vector.tensor_mul(sc, rz, gbc)  # gamma / Z
                    scales.append(sc)
                st[s]["scales"] = scales

            def expTf(s):
                expn = st[s]["expn"]
                expT = []
                for j in range(CB):
                    bounce = ps_t.tile([128, CB, 128], F16, tag="bounce",
                                       name=f"ebounce_{s}_{j}")
                    for cb in range(CB):
                        nc.tensor.transpose(
                            bounce[:, cb, :],
                            expn[:, cb, j * 128 : (j + 1) * 128],
                            ident,
                        )
                    et = exptp.tile([128, CB, 128], F16, tag="expT",
                                    name=f"expT_{s}_{j}")
                    if j % 2 == 0:
                        nc.scalar.copy(et[:], bounce[:, :, :])
                    else:
                        nc.vector.tensor_copy(et[:], bounce[:, :, :])
                    expT.append(et)
                st[s]["expT"] = expT

            def aphase(s, lo=0, hi=CB * NO):
                # out = gamma/Z * (exp @ q) + x, staged f16 into 512KB DMAs
                q16, expT, scales = st[s]["q16"], st[s]["expT"], st[s]["scales"]
                ostage = st[s].setdefault("ostage", {})
                # cb-major so each c-block's staged stores fire as soon as
                # its row finishes; the final stage is split in two for a
                # shorter drain after the last matmul
                for ci in range(lo, hi):
                    cb, no = ci // NO, ci % NO
                    if True:
                        npc, nof = no // (PW // 512), (no % (PW // 512)) * 512
                        psl = slice(nof, nof + 512)
                        acc = ps_o.tile([128, 512], F32, tag="acc",
                                        name=f"acc_{s}_{no}_{cb}")
                        for j in range(CB):
                            nc.tensor.matmul(
                                acc[:],
                                lhsT=expT[j][:, cb, :],
                                rhs=q16[j][npc][:, psl],
                                start=(j == 0),
                                stop=(j == CB - 1),
                            )
                        half = no // (NO // 2)
                        if (cb, half) not in ostage:
                            ot = outsp.tile([128, (NO // 2) * 512], F16,
                                            tag="ot", name=f"ot_{s}_{cb}_{half}")
                            ostage[(cb, half)] = ot
                        ot = ostage[(cb, half)]
                        osl = slice((no % (NO // 2)) * 512,
                                    (no % (NO // 2) + 1) * 512)
                        nc.vector.scalar_tensor_tensor(
                            out=ot[:, osl],
                            in0=acc[:],
                            scalar=scales[cb],
                            in1=q16[cb][npc][:, psl],
                            op0=mybir.AluOpType.mult,
                            op1=mybir.AluOpType.add,
                        )
                        rows = slice(s * C + cb * 128, s * C + (cb + 1) * 128)
                        base = half * (NO // 2) * 512
                        last = (s == 1 and cb == CB - 1 and half == 1)
                        if last:
                            hh = no % (NO // 2)
                            nc.sync.dma_start(
                                out=out_ap[rows, base + hh * 512 :
                                           base + (hh + 1) * 512],
                                in_=ot[:, hh * 512 : (hh + 1) * 512],
                            )
                        elif not last and no % (NO // 2) == NO // 2 - 1:
                            nc.sync.dma_start(
                                out=out_ap[rows, base : base + (NO // 2) * 512],
                                in_=ot[:],
                            )

            # ---- interleaved emission schedule -----------------------
            load(0)
            for g in range(NG):
                tgroup(0, g)
                emm(0, g)
            # sample-1 loads/casts/first transposes outrank softmax(0) so
            # DVE/ACT feed the PE through the softmax window; softmax(0)
            # has slack since A(0) runs only after E(1)
            load(1)
            for g in range(PRE):
                tgroup(1, g)
            softmax(0)
            expTf(0)
            for g in range(PRE):
                emm(1, g)
            for g in range(PRE, NG):
                tgroup(1, g)
                emm(1, g)
            # A(s0) emitted after E(s1); sample-1's softmax chain is
            # spread through a0's tail chunks so it hides completely:
            # copies/exp on ACT (idle during a0), rowmins rationed so
            # DVE's STT stream isn't starved
            aphase(0, 0, 20)
            softmax_esb_mirror(1)
            aphase(0, 20, 24)
            softmax_rowmin(1, [0, 1])
            aphase(0, 24, 26)
            softmax_rowmin(1, [2, 3])
            softmax_exp(1, [0, 1])
            aphase(0, 26, 28)
            softmax_exp(1, [2, 3])
            softmax_scales(1)
            aphase(0, 28, 32)
            expTf(1)
            aphase(1)
    return nc


def _split_excess_waits(nc, max_waits=1):
    """This container's walrus rejects >1 sync-wait on one instruction
    ("Too many sync wait commands"); hoist extras onto standalone
    InstEventSemaphore preludes on the same engine."""
    n = 0
    for fn in nc.m.functions:
        for bb in fn.blocks:
            out = []
            for inst in bb.instructions:
                si = inst.sync_info
                if si is not None and si.on_wait and len(si.on_wait) > max_waits:
                    waits = list(si.on_wait)
                    head, keep = waits[:-max_waits], waits[-max_waits:]
                    for i, w in enumerate(head):
                        ev = mybir.InstEventSemaphore(
                            name=f"{inst.name}-wsplit{i}", ins=[], outs=[])
                        ev.engine = inst.engine
                        ev.sync_info = mybir.SyncInfo(on_wait=[w], on_update=[])
                        out.append(ev)
                        n += 1
                    inst.sync_info = mybir.SyncInfo(
                        on_wait=keep, on_update=list(si.on_update))
                out.append(inst)
            bb.instructions[:] = out
    return n


_cache = {}


def _get_nc():
    if 'nc' not in _cache:
        nc = bass.Bass()
        build(nc)
        _split_excess_waits(nc)
        _cache['nc'] = nc
    return _cache['nc']


def kernel(x: np.ndarray, gamma: np.ndarray) -> np.ndarray:
    from concourse.bass_utils import run_bass_kernel_spmd

    B, CH, H, W = x.shape          # (16, 512, 64, 64)
    NSP = H * W
    M = 8                          # cores
    SS = B // M                    # samples per core
    nc = _get_nc()
    g = np.ascontiguousarray(gamma, dtype=np.float32).reshape(1, 1)
    in_maps = [
        {
            "x": np.ascontiguousarray(
                x[i * SS : (i + 1) * SS].reshape(SS * CH, NSP), dtype=np.float32
            ),
            "gamma": g,
        }
        for i in range(M)
    ]
    res = run_bass_kernel_spmd(nc, in_maps, core_ids=list(range(M)))
    out = np.concatenate(
        [res.results[i]["out"].astype(np.float32).reshape(SS, CH, H, W)
         for i in range(M)],
        axis=0,
    )
    return np.ascontiguousarray(out, dtype=np.float32)


# revision 19
# speedup vs baseline: 1.1014x; 1.1014x over previous
"""Self-contained TRN2 Bass kernel for nn_CAM_Module (channel attention).

kernel(x, gamma): x [16,512,64,64] f32, gamma [1] f32 -> [16,512,64,64] f32.
Data-parallel over batch: 2 samples per NeuronCore across 8 cores.

Math: q = x.reshape(B,C,HW); E = q@q.T; softmax(rowmax(E)-E) == softmax(-E)
(shift invariance), computed as exp(rowmin(E)-E)/rowsum; out = gamma*(A@q)+x.

On-chip strategy (per core, 2 samples):
  - load fp32, cast to fp16 (DVE/ACT), PE-transpose 128x128 tiles to build
    q^T chunks; single-pass fp16 Gram accumulated in fp32 PSUM, computing
    only upper-triangle blocks (E symmetric), mirrored via PE transposes.
  - softmax via ACT exp(scale=-1, bias=rowmin) with fused rowsum accum_out.
  - A-matmul fp16: lhsT = transpose(exp), rhs = fp16(q); epilogue fused in
    one DVE scalar_tensor_tensor: out = psum*(gamma/rowsum) + fp16(x),
    written as fp16 (output tensor stored f16 to halve output HBM traffic;
    host converts back to f32 -- costs ~2e-4 relative quantization noise).
"""
import sys
if '/opt/trn_rl_repo' not in sys.path:
    sys.path.insert(0, '/opt/trn_rl_repo')
import numpy as np
import concourse.bass as bass
import concourse.tile as tile
import concourse.mybir as mybir
from concourse.masks import make_identity

F32 = mybir.dt.float32
F16 = mybir.dt.float16

C = 512          # channels
N = 4096         # spatial (64*64)
CB = C // 128    # 4 c-blocks
NK = N // 128    # 32 transpose chunks
NG = NK // 2     # 16 transpose groups (2 chunks per PSUM bounce bank)
NO = N // 512    # 8 output column chunks
NP = 8          # load/cast pieces per row-block (512 cols each)
PW = N // NP     # piece width
S = 2            # samples per core
PRE = 8          # sample-1 transpose groups hoisted before sample-0 softmax


def build(nc: bass.Bass):
    x_ext = nc.declare_dram_parameter("x", [S * C, N], F32, isOutput=False)
    g_ext = nc.declare_dram_parameter("gamma", [1, 1], F32, isOutput=False)
    out_ext = nc.declare_dram_parameter("out", [S * C, N], F16, isOutput=True)
    x_ap = x_ext.ap()
    out_ap = out_ext.ap()
    EW = [512 - 128 * m for m in range(CB)]

    with tile.TileContext(nc) as tc:
        with (
            tc.tile_pool(name="const", bufs=1) as const,
            tc.tile_pool(name="x32", bufs=12) as x32,
            tc.tile_pool(name="q16", bufs=2 * CB * NP) as q16p,
            tc.tile_pool(name="qt", bufs=14) as qtp,
            tc.tile_pool(name="esb", bufs=2) as esbp,
            tc.tile_pool(name="expn", bufs=2) as expnp,
            tc.tile_pool(name="expt", bufs=2 * CB) as exptp,
            tc.tile_pool(name="vecs", bufs=4 * CB) as vecs,
            tc.tile_pool(name="outs", bufs=7) as outsp,
            tc.tile_pool(name="ps_bounce", bufs=2, space="PSUM") as ps_t,
            tc.tile_pool(name="ps_e", bufs=1, space="PSUM") as ps_e,
            tc.tile_pool(name="ps_o", bufs=2, space="PSUM") as ps_o,
        ):
            ident = const.tile([128, 128], F16)
            make_identity(nc, ident)
            ident32 = const.tile([128, 128], F32)
            make_identity(nc, ident32)
            gbc = const.tile([128, 1], F32)
            nc.gpsimd.dma_start(out=gbc, in_=g_ext.ap().to_broadcast((128, 1)))

            st = [dict() for _ in range(S)]

            def load(s):
                # 512KB pieces so compute starts early; q16[cb][p] covers
                # columns [p*PW, (p+1)*PW)
                q16 = [[None] * NP for _ in range(CB)]
                for p in range(NP):
                    for cb in range(CB):
                        qc = q16p.tile([128, PW], F16, tag="q16",
                                       name=f"q16_{s}_{cb}_{p}")
                        if p == 0 and s == 0:
                            # first piece in 256-col halves so the very
                            # first transposes start sooner
                            for hh in range(2):
                                xt = x32.tile([128, PW // 2], F32, tag="xth",
                                              bufs=4,
                                              name=f"xt_{s}_{cb}_{p}_{hh}")
                                nc.sync.dma_start(
                                    out=xt,
                                    in_=x_ap[
                                        s * C + cb * 128
                                        : s * C + (cb + 1) * 128,
                                        hh * (PW // 2) : (hh + 1) * (PW // 2),
                                    ],
                                )
                                dst = qc[:, hh * (PW // 2) :
                                         (hh + 1) * (PW // 2)]
                                if (cb + hh) % 2 == 0:
                                    nc.vector.tensor_copy(dst, xt[:])
                                else:
                                    nc.scalar.copy(dst, xt[:])
                            q16[cb][p] = qc
                            continue
                        xt = x32.tile([128, PW], F32, tag="xt",
                                      name=f"xt_{s}_{cb}_{p}")
                        nc.sync.dma_start(
                            out=xt,
                            in_=x_ap[
                                s * C + cb * 128 : s * C + (cb + 1) * 128,
                                p * PW : (p + 1) * PW,
                            ],
                        )
                        if (p * CB + cb) % 2 == 0:
                            nc.vector.tensor_copy(qc[:], xt[:])
                        else:
                            nc.scalar.copy(qc[:], xt[:])
                        q16[cb][p] = qc
                st[s]["q16"] = q16
                st[s]["qtc"] = {}

            def tgroup(s, g):
                # transpose 2 chunks (8 [128,128] fp16 tiles) into one
                # PSUM bounce bank, evacuate to SBUF in one op
                q16 = st[s]["q16"]
                # odd groups bounce through the (idle-during-Gram) ps_o
                # pool: an effective 4-deep transpose ring, so T0 of
                # group g waits evac(g-4) instead of evac(g-2)
                pool, tag = (ps_t, "bounce") if g % 2 == 0 else (ps_o, "acc")
                bounce = pool.tile([128, 2, CB, 128], F16, tag=tag,
                                   name=f"bounce_{s}_{g}")
                for h in range(2):
                    k = 2 * g + h
                    kp, ko = k // (PW // 128), (k % (PW // 128)) * 128
                    for cb in range(CB):
                        nc.tensor.transpose(
                            bounce[:, h, cb, :],
                            q16[cb][kp][:, ko : ko + 128],
                            ident,
                        )
                qtc = qtp.tile([128, 2, CB * 128], F16, tag="qtc",
                               name=f"qtc_{s}_{g}")
                if g % 2 == 0:
                    nc.scalar.copy(qtc[:], bounce[:, :, :, :])
                else:
                    nc.vector.tensor_copy(qtc[:], bounce[:, :, :, :])
                st[s]["qtc"][g] = qtc

            def emm(s, g):
                # symmetric Gram accumulation: upper-triangle blocks only
                if "E" not in st[s]:
                    st[s]["E"] = ps_e.tile([128, CB, 512], F32, tag="E",
                                           name=f"E_{s}")
                E = st[s]["E"]
                qtc = st[s]["qtc"][g]
                for h in range(2):
                    k = 2 * g + h
                    for m in range(CB):
                        nc.tensor.matmul(
                            E[:, m, 0 : EW[m]],
                            lhsT=qtc[:, h, m * 128 : (m + 1) * 128],
                            rhs=qtc[:, h, m * 128 : 512],
                            start=(k == 0),
                            stop=(k == NK - 1),
                        )

            def softmax(s):
                # rebuild full E rows in SBUF (mirror lower triangle),
                # then exp(rowmin - E) with fused rowsum
                E = st[s]["E"]
                E_sb = esbp.tile([128, CB, 512], F32, tag="esb",
                                 name=f"esb_{s}")
                for m in range(CB):
                    if m % 2 == 0:
                        nc.scalar.copy(E_sb[:, m, m * 128 : 512],
                                       E[:, m, 0 : EW[m]])
                    else:
                        nc.vector.tensor_copy(E_sb[:, m, m * 128 : 512],
                                              E[:, m, 0 : EW[m]])
                for i in range(CB):
                    for j in range(i):
                        tb = ps_o.tile([128, 128], F32, tag="acc",
                                       name=f"tb_{s}_{i}_{j}")
                        nc.tensor.transpose(
                            tb[:], E_sb[:, j, i * 128 : (i + 1) * 128], ident32
                        )
                        if (i + j) % 2 == 0:
                            nc.scalar.copy(
                                E_sb[:, i, j * 128 : (j + 1) * 128], tb[:])
                        else:
                            nc.vector.tensor_copy(
                                E_sb[:, i, j * 128 : (j + 1) * 128], tb[:])
                expn = expnp.tile([128, CB, 512], F16, tag="expn",
                                  name=f"expn_{s}")
                scales = []
                for m in range(CB):
                    mv = vecs.tile([128, 1], F32, tag="mv", name=f"mv_{s}_{m}")
                    nc.vector.tensor_reduce(
                        mv, E_sb[:, m, :], axis=mybir.AxisListType.X,
                        op=mybir.AluOpType.min,
                    )
                    Z = vecs.tile([128, 1], F32, tag="Z", name=f"Z_{s}_{m}")
                    nc.scalar.activation(
                        expn[:, m, :],
                        E_sb[:, m, :],
                        mybir.ActivationFunctionType.Exp,
                        bias=mv,
                        scale=-1.0,
                        accum_out=Z,
                    )
                    rz = vecs.tile([128, 1], F32, tag="rz", name=f"rz_{s}_{m}")
                    nc.vector.reciprocal(rz, Z)
                    sc = vecs.tile([128, 1], F32, tag="sc", name=f"sc_{s}_{m}")
                    nc.vector.tensor_mul(sc, rz, gbc)  # gamma / Z
                    scales.append(sc)
                st[s]["expn"] = expn
                st[s]["scales"] = scales

            def softmax_esb_mirror(s):
                # E_sb rebuild with copies on ACT (DVE is busy with the
                # a0 epilogue STTs this runs under)
                E = st[s]["E"]
                E_sb = esbp.tile([128, CB, 512], F32, tag="esb",
                                 name=f"esb_{s}")
                for m in range(CB):
                    nc.scalar.copy(E_sb[:, m, m * 128 : 512],
                                   E[:, m, 0 : EW[m]])
                for i in range(CB):
                    for j in range(i):
                        tb = ps_o.tile([128, 128], F32, tag="acc",
                                       name=f"tb_{s}_{i}_{j}")
                        nc.tensor.transpose(
                            tb[:], E_sb[:, j, i * 128 : (i + 1) * 128], ident32
                        )
                        nc.scalar.copy(
                            E_sb[:, i, j * 128 : (j + 1) * 128], tb[:])
                st[s]["E_sb"] = E_sb

            def softmax_rowmin(s, ms):
                E_sb = st[s]["E_sb"]
                mvs = st[s].setdefault("mvs", {})
                for m in ms:
                    mv = vecs.tile([128, 1], F32, tag="mv", name=f"mv_{s}_{m}")
                    nc.vector.tensor_reduce(
                        mv, E_sb[:, m, :], axis=mybir.AxisListType.X,
                        op=mybir.AluOpType.min,
                    )
                    mvs[m] = mv

            def softmax_exp(s, ms):
                E_sb = st[s]["E_sb"]
                expn = st[s].setdefault(
                    "expn",
                    expnp.tile([128, CB, 512], F16, tag="expn",
                               name=f"expn_{s}"))
                Zs = st[s].setdefault("Zs", {})
                for m in ms:
                    Z = vecs.tile([128, 1], F32, tag="Z", name=f"Z_{s}_{m}")
                    nc.scalar.activation(
                        expn[:, m, :],
                        E_sb[:, m, :],
                        mybir.ActivationFunctionType.Exp,
                        bias=st[s]["mvs"][m],
                        scale=-1.0,
                        accum_out=Z,
                    )
                    Zs[m] = Z

            def softmax_scales(s):
                scales = []
                for m in range(CB):
                    rz = vecs.tile([128, 1], F32, tag="rz", name=f"rz_{s}_{m}")
                    nc.vector.reciprocal(rz, st[s]["Zs"][m])
                    sc = vecs.tile([128, 1], F32, tag="sc", name=f"sc_{s}_{m}")
                    nc.vector.tensor_mul(sc, rz, gbc)  # gamma / Z
                    scales.append(sc)
                st[s]["scales"] = scales

            def expTf(s):
                expn = st[s]["expn"]
                expT = []
                for j in range(CB):
                    bounce = ps_t.tile([128, CB, 128], F16, tag="bounce",
                                       name=f"ebounce_{s}_{j}")
                    for cb in range(CB):
                        nc.tensor.transpose(
                            bounce[:, cb, :],
                            expn[:, cb, j * 128 : (j + 1) * 128],
                            ident,
                        )
                    et = exptp.tile([128, CB, 128], F16, tag="expT",
                                    name=f"expT_{s}_{j}")
                    if j % 2 == 0:
                        nc.scalar.copy(et[:], bounce[:, :, :])
                    else:
                        nc.vector.tensor_copy(et[:], bounce[:, :, :])
                    expT.append(et)
                st[s]["expT"] = expT

            def aphase(s, lo=0, hi=CB * NO):
                # out = gamma/Z * (exp @ q) + x, staged f16 into 512KB DMAs
                q16, expT, scales = st[s]["q16"], st[s]["expT"], st[s]["scales"]
                ostage = st[s].setdefault("ostage", {})
                # cb-major so each c-block's staged stores fire as soon as
                # its row finishes; the final stage is split in two for a
                # shorter drain after the last matmul
                for ci in range(lo, hi):
                    cb, no = ci // NO, ci % NO
                    if True:
                        npc, nof = no // (PW // 512), (no % (PW // 512)) * 512
                        psl = slice(nof, nof + 512)
                        acc = ps_o.tile([128, 512], F32, tag="acc",
                                        name=f"acc_{s}_{no}_{cb}")
                        for j in range(CB):
                            nc.tensor.matmul(
                                acc[:],
                                lhsT=expT[j][:, cb, :],
                                rhs=q16[j][npc][:, psl],
                                start=(j == 0),
                                stop=(j == CB - 1),
                            )
                        half = no // (NO // 2)
                        if (cb, half) not in ostage:
                            ot = outsp.tile([128, (NO // 2) * 512], F16,
                                            tag="ot", name=f"ot_{s}_{cb}_{half}")
                            ostage[(cb, half)] = ot
                        ot = ostage[(cb, half)]
                        osl = slice((no % (NO // 2)) * 512,
                                    (no % (NO // 2) + 1) * 512)
                        nc.vector.scalar_tensor_tensor(
                            out=ot[:, osl],
                            in0=acc[:],
                            scalar=scales[cb],
                            in1=q16[cb][npc][:, psl],
                            op0=mybir.AluOpType.mult,
                            op1=mybir.AluOpType.add,
                        )
                        rows = slice(s * C + cb * 128, s * C + (cb + 1) * 128)
                        base = half * (NO // 2) * 512
                        last = (s == 1 and cb == CB - 1 and half == 1)
                        if last:
                            hh = no % (NO // 2)
                            nc.sync.dma_start(
                                out=out_ap[rows, base + hh * 512 :
                                           base + (hh + 1) * 512],
                                in_=ot[:, hh * 512 : (hh + 1) * 512],
                            )
                        elif not last and no % (NO // 2) == NO // 2 - 1:
                            nc.sync.dma_start(
                                out=out_ap[rows, base : base + (NO // 2) * 512],
                                in_=ot[:],
                            )

            # ---- interleaved emission schedule -----------------------
            load(0)
            for g in range(NG):
                tgroup(0, g)
                emm(0, g)
            # sample-1 loads/casts/first transposes outrank softmax(0) so
            # DVE/ACT feed the PE through the softmax window; softmax(0)
            # has slack since A(0) runs only after E(1)
            load(1)
            for g in range(PRE):
                tgroup(1, g)
            softmax(0)
            expTf(0)
            for g in range(PRE):
                emm(1, g)
            for g in range(PRE, NG):
                tgroup(1, g)
                emm(1, g)
            # A(s0) emitted after E(s1); sample-1's softmax chain is
            # spread through a0's tail chunks so it hides completely:
            # copies/exp on ACT (idle during a0), rowmins rationed so
            # DVE's STT stream isn't starved
            aphase(0, 0, 20)
            softmax_esb_mirror(1)
            aphase(0, 20, 24)
            softmax_rowmin(1, [0, 1])
            aphase(0, 24, 26)
            softmax_rowmin(1, [2, 3])
            softmax_exp(1, [0, 1])
            aphase(0, 26, 28)
            softmax_exp(1, [2, 3])
            softmax_scales(1)
            aphase(0, 28, 32)
            expTf(1)
            aphase(1)
    return nc


def _split_excess_waits(nc, max_waits=1):
    """This container's walrus rejects >1 sync-wait on one instruction
    ("Too many sync wait commands"); hoist extras onto standalone
    InstEventSemaphore preludes on the same engine."""
    n = 0
    for fn in nc.m.functions:
        for bb in fn.blocks:
            out = []
            for inst in bb.instructions:
                si = inst.sync_info
                if si is not None and si.on_wait and len(si.on_wait) > max_waits:
                    waits = list(si.on_wait)
                    head, keep = waits[:-max_waits], waits[-max_waits:]
                    for i, w in enumerate(head):
                        ev = mybir.InstEventSemaphore(
                            name=f"{inst.name}-wsplit{i}", ins=[], outs=[])
                        ev.engine = inst.engine
                        ev.sync_info = mybir.SyncInfo(on_wait=[w], on_update=[])
                        out.append(ev)
                        n += 1
                    inst.sync_info = mybir.SyncInfo(
                        on_wait=keep, on_update=list(si.on_update))
                out.append(inst)
            bb.instructions[:] = out
    return n


_cache = {}


def _get_nc():
    if 'nc' not in _cache:
        nc = bass.Bass()
        build(nc)
        _split_excess_waits(nc)
        _cache['nc'] = nc
    return _cache['nc']


def kernel(x: np.ndarray, gamma: np.ndarray) -> np.ndarray:
    from concourse.bass_utils import run_bass_kernel_spmd

    B, CH, H, W = x.shape          # (16, 512, 64, 64)
    NSP = H * W
    M = 8                          # cores
    SS = B // M                    # samples per core
    nc = _get_nc()
    g = np.ascontiguousarray(gamma, dtype=np.float32).reshape(1, 1)
    in_maps = [
        {
            "x": np.ascontiguousarray(
                x[i * SS : (i + 1) * SS].reshape(SS * CH, NSP), dtype=np.float32
            ),
            "gamma": g,
        }
        for i in range(M)
    ]
    res = run_bass_kernel_spmd(nc, in_maps, core_ids=list(range(M)))
    out = np.concatenate(
        [res.results[i]["out"].astype(np.float32).reshape(SS, CH, H, W)
         for i in range(M)],
        axis=0,
    )
    return np.ascontiguousarray(out, dtype=np.float32)


# revision 20
# speedup vs baseline: 1.1329x; 1.0286x over previous
"""Self-contained TRN2 Bass kernel for nn_CAM_Module (channel attention).

kernel(x, gamma): x [16,512,64,64] f32, gamma [1] f32 -> [16,512,64,64] f32.
Data-parallel over batch: 2 samples per NeuronCore across 8 cores.

Math: q = x.reshape(B,C,HW); E = q@q.T; softmax(rowmax(E)-E) == softmax(-E)
(shift invariance), computed as exp(rowmin(E)-E)/rowsum; out = gamma*(A@q)+x.

On-chip strategy (per core, 2 samples):
  - load fp32, cast to fp16 (DVE/ACT), PE-transpose 128x128 tiles to build
    q^T chunks; single-pass fp16 Gram accumulated in fp32 PSUM, computing
    only upper-triangle blocks (E symmetric), mirrored via PE transposes.
  - softmax via ACT exp(scale=-1, bias=rowmin) with fused rowsum accum_out.
  - A-matmul fp16: lhsT = transpose(exp), rhs = fp16(q); epilogue fused in
    one DVE scalar_tensor_tensor: out = psum*(gamma/rowsum) + fp16(x),
    written as fp16 (output tensor stored f16 to halve output HBM traffic;
    host converts back to f32 -- costs ~2e-4 relative quantization noise).
"""
import sys
if '/opt/trn_rl_repo' not in sys.path:
    sys.path.insert(0, '/opt/trn_rl_repo')
import numpy as np
import concourse.bass as bass
import concourse.tile as tile
import concourse.mybir as mybir
from concourse.masks import make_identity

F32 = mybir.dt.float32
F16 = mybir.dt.float16

C = 512          # channels
N = 4096         # spatial (64*64)
CB = C // 128    # 4 c-blocks
NK = N // 128    # 32 transpose chunks
NG = NK // 2     # 16 transpose groups (2 chunks per PSUM bounce bank)
NO = N // 512    # 8 output column chunks
NP = 8          # load/cast pieces per row-block (512 cols each)
PW = N // NP     # piece width
S = 2            # samples per core
PRE = 8          # sample-1 transpose groups hoisted before sample-0 softmax


def build(nc: bass.Bass):
    x_ext = nc.declare_dram_parameter("x", [S * C, N], F32, isOutput=False)
    g_ext = nc.declare_dram_parameter("gamma", [1, 1], F32, isOutput=False)
    out_ext = nc.declare_dram_parameter("out", [S * C, N], F16, isOutput=True)
    x_ap = x_ext.ap()
    out_ap = out_ext.ap()
    EW = [512 - 128 * m for m in range(CB)]

    with tile.TileContext(nc) as tc:
        with (
            tc.tile_pool(name="const", bufs=1) as const,
            tc.tile_pool(name="x32", bufs=12) as x32,
            tc.tile_pool(name="q16", bufs=2 * CB * NP) as q16p,
            tc.tile_pool(name="qt", bufs=14) as qtp,
            tc.tile_pool(name="esb", bufs=2) as esbp,
            tc.tile_pool(name="expn", bufs=2) as expnp,
            tc.tile_pool(name="expt", bufs=2 * CB) as exptp,
            tc.tile_pool(name="vecs", bufs=4 * CB) as vecs,
            tc.tile_pool(name="outs", bufs=7) as outsp,
            tc.tile_pool(name="ps_bounce", bufs=2, space="PSUM") as ps_t,
            tc.tile_pool(name="ps_e", bufs=1, space="PSUM") as ps_e,
            tc.tile_pool(name="ps_o", bufs=2, space="PSUM") as ps_o,
        ):
            ident = const.tile([128, 128], F16)
            make_identity(nc, ident)
            ident32 = const.tile([128, 128], F32)
            make_identity(nc, ident32)
            gbc = const.tile([128, 1], F32)
            nc.gpsimd.dma_start(out=gbc, in_=g_ext.ap().to_broadcast((128, 1)))

            st = [dict() for _ in range(S)]

            def load(s):
                # 512KB pieces so compute starts early; q16[cb][p] covers
                # columns [p*PW, (p+1)*PW)
                q16 = [[None] * NP for _ in range(CB)]
                for p in range(NP):
                    for cb in range(CB):
                        qc = q16p.tile([128, PW], F16, tag="q16",
                                       name=f"q16_{s}_{cb}_{p}")
                        if p == 0 and s == 0:
                            # first piece in 256-col halves so the very
                            # first transposes start sooner
                            for hh in range(2):
                                xt = x32.tile([128, PW // 2], F32, tag="xth",
                                              bufs=4,
                                              name=f"xt_{s}_{cb}_{p}_{hh}")
                                nc.sync.dma_start(
                                    out=xt,
                                    in_=x_ap[
                                        s * C + cb * 128
                                        : s * C + (cb + 1) * 128,
                                        hh * (PW // 2) : (hh + 1) * (PW // 2),
                                    ],
                                )
                                dst = qc[:, hh * (PW // 2) :
                                         (hh + 1) * (PW // 2)]
                                if (cb + hh) % 2 == 0:
                                    nc.vector.tensor_copy(dst, xt[:])
                                else:
                                    nc.scalar.copy(dst, xt[:])
                            q16[cb][p] = qc
                            continue
                        xt = x32.tile([128, PW], F32, tag="xt",
                                      name=f"xt_{s}_{cb}_{p}")
                        nc.sync.dma_start(
                            out=xt,
                            in_=x_ap[
                                s * C + cb * 128 : s * C + (cb + 1) * 128,
                                p * PW : (p + 1) * PW,
                            ],
                        )
                        if (p * CB + cb) % 2 == 0:
                            nc.vector.tensor_copy(qc[:], xt[:])
                        else:
                            nc.scalar.copy(qc[:], xt[:])
                        q16[cb][p] = qc
                st[s]["q16"] = q16
                st[s]["qtc"] = {}

            def tgroup(s, g):
                # transpose 2 chunks (8 [128,128] fp16 tiles) into one
                # PSUM bounce bank, evacuate to SBUF in one op
                q16 = st[s]["q16"]
                # odd groups bounce through the (idle-during-Gram) ps_o
                # pool: an effective 4-deep transpose ring, so T0 of
                # group g waits evac(g-4) instead of evac(g-2)
                pool, tag = (ps_t, "bounce") if g % 2 == 0 else (ps_o, "acc")
                bounce = pool.tile([128, 2, CB, 128], F16, tag=tag,
                                   name=f"bounce_{s}_{g}")
                for h in range(2):
                    k = 2 * g + h
                    kp, ko = k // (PW // 128), (k % (PW // 128)) * 128
                    for cb in range(CB):
                        nc.tensor.transpose(
                            bounce[:, h, cb, :],
                            q16[cb][kp][:, ko : ko + 128],
                            ident,
                        )
                qtc = qtp.tile([128, 2, CB * 128], F16, tag="qtc",
                               name=f"qtc_{s}_{g}")
                if g % 2 == 0:
                    nc.scalar.copy(qtc[:], bounce[:, :, :, :])
                else:
                    nc.vector.tensor_copy(qtc[:], bounce[:, :, :, :])
                st[s]["qtc"][g] = qtc

            def emm(s, g):
                # symmetric Gram accumulation: upper-triangle blocks only
                if "E" not in st[s]:
                    st[s]["E"] = ps_e.tile([128, CB, 512], F32, tag="E",
                                           name=f"E_{s}")
                E = st[s]["E"]
                qtc = st[s]["qtc"][g]
                for h in range(2):
                    k = 2 * g + h
                    for m in range(CB):
                        nc.tensor.matmul(
                            E[:, m, 0 : EW[m]],
                            lhsT=qtc[:, h, m * 128 : (m + 1) * 128],
                            rhs=qtc[:, h, m * 128 : 512],
                            start=(k == 0),
                            stop=(k == NK - 1),
                        )

            def softmax(s):
                # rebuild full E rows in SBUF (mirror lower triangle),
                # then exp(rowmin - E) with fused rowsum
                E = st[s]["E"]
                E_sb = esbp.tile([128, CB, 512], F32, tag="esb",
                                 name=f"esb_{s}")
                for m in range(CB):
                    if m % 2 == 0:
                        nc.scalar.copy(E_sb[:, m, m * 128 : 512],
                                       E[:, m, 0 : EW[m]])
                    else:
                        nc.vector.tensor_copy(E_sb[:, m, m * 128 : 512],
                                              E[:, m, 0 : EW[m]])
                for i in range(CB):
                    for j in range(i):
                        tb = ps_o.tile([128, 128], F32, tag="acc",
                                       name=f"tb_{s}_{i}_{j}")
                        nc.tensor.transpose(
                            tb[:], E_sb[:, j, i * 128 : (i + 1) * 128], ident32
                        )
                        if (i + j) % 2 == 0:
                            nc.scalar.copy(
                                E_sb[:, i, j * 128 : (j + 1) * 128], tb[:])
                        else:
                            nc.vector.tensor_copy(
                                E_sb[:, i, j * 128 : (j + 1) * 128], tb[:])
                expn = expnp.tile([128, CB, 512], F16, tag="expn",
                                  name=f"expn_{s}")
                scales = []
                for m in range(CB):
                    mv = vecs.tile([128, 1], F32, tag="mv", name=f"mv_{s}_{m}")
                    nc.vector.tensor_reduce(
                        mv, E_sb[:, m, :], axis=mybir.AxisListType.X,
                        op=mybir.AluOpType.min,
                    )
                    Z = vecs.tile([128, 1], F32, tag="Z", name=f"Z_{s}_{m}")
                    nc.scalar.activation(
                        expn[:, m, :],
                        E_sb[:, m, :],
                        mybir.ActivationFunctionType.Exp,
                        bias=mv,
                        scale=-1.0,
                        accum_out=Z,
                    )
                    rz = vecs.tile([128, 1], F32, tag="rz", name=f"rz_{s}_{m}")
                    nc.vector.reciprocal(rz, Z)
                    sc = vecs.tile([128, 1], F32, tag="sc", name=f"sc_{s}_{m}")
                    nc.vector.tensor_mul(sc, rz, gbc)  # gamma / Z
                    scales.append(sc)
                st[s]["expn"] = expn
                st[s]["scales"] = scales

            def softmax_esb_mirror(s):
                # E_sb rebuild with copies on ACT (DVE is busy with the
                # a0 epilogue STTs this runs under)
                E = st[s]["E"]
                E_sb = esbp.tile([128, CB, 512], F32, tag="esb",
                                 name=f"esb_{s}")
                for m in range(CB):
                    nc.scalar.copy(E_sb[:, m, m * 128 : 512],
                                   E[:, m, 0 : EW[m]])
                for i in range(CB):
                    for j in range(i):
                        tb = ps_o.tile([128, 128], F32, tag="acc",
                                       name=f"tb_{s}_{i}_{j}")
                        nc.tensor.transpose(
                            tb[:], E_sb[:, j, i * 128 : (i + 1) * 128], ident32
                        )
                        nc.scalar.copy(
                            E_sb[:, i, j * 128 : (j + 1) * 128], tb[:])
                st[s]["E_sb"] = E_sb

            def softmax_rowmin(s, ms):
                E_sb = st[s]["E_sb"]
                mvs = st[s].setdefault("mvs", {})
                for m in ms:
                    mv = vecs.tile([128, 1], F32, tag="mv", name=f"mv_{s}_{m}")
                    nc.vector.tensor_reduce(
                        mv, E_sb[:, m, :], axis=mybir.AxisListType.X,
                        op=mybir.AluOpType.min,
                    )
                    mvs[m] = mv

            def softmax_exp(s, ms):
                E_sb = st[s]["E_sb"]
                expn = st[s].setdefault(
                    "expn",
                    expnp.tile([128, CB, 512], F16, tag="expn",
                               name=f"expn_{s}"))
                Zs = st[s].setdefault("Zs", {})
                for m in ms:
                    Z = vecs.tile([128, 1], F32, tag="Z", name=f"Z_{s}_{m}")
                    nc.scalar.activation(
                        expn[:, m, :],
                        E_sb[:, m, :],
                        mybir.ActivationFunctionType.Exp,
                        bias=st[s]["mvs"][m],
                        scale=-1.0,
                        accum_out=Z,
                    )
                    Zs[m] = Z

            def softmax_scales(s):
                scales = []
                for m in range(CB):
                    rz = vecs.tile([128, 1], F32, tag="rz", name=f"rz_{s}_{m}")
                    nc.vector.reciprocal(rz, st[s]["Zs"][m])
                    sc = vecs.tile([128, 1], F32, tag="sc", name=f"sc_{s}_{m}")
                    nc.vector.tensor_mul(sc, rz, gbc)  # gamma / Z
                    scales.append(sc)
                st[s]["scales"] = scales

            def expTf(s):
                expn = st[s]["expn"]
                expT = []
                for j in range(CB):
                    bounce = ps_t.tile([128, CB, 128], F16, tag="bounce",
                                       name=f"ebounce_{s}_{j}")
                    for cb in range(CB):
                        nc.tensor.transpose(
                            bounce[:, cb, :],
                            expn[:, cb, j * 128 : (j + 1) * 128],
                            ident,
                        )
                    et = exptp.tile([128, CB, 128], F16, tag="expT",
                                    name=f"expT_{s}_{j}")
                    if j % 2 == 0:
                        nc.scalar.copy(et[:], bounce[:, :, :])
                    else:
                        nc.vector.tensor_copy(et[:], bounce[:, :, :])
                    expT.append(et)
                st[s]["expT"] = expT

            def aphase(s, lo=0, hi=CB * NO):
                # out = gamma/Z * (exp @ q) + x, staged f16 into 512KB DMAs
                q16, expT, scales = st[s]["q16"], st[s]["expT"], st[s]["scales"]
                ostage = st[s].setdefault("ostage", {})
                # cb-major so each c-block's staged stores fire as soon as
                # its row finishes; the final stage is split in two for a
                # shorter drain after the last matmul
                for ci in range(lo, hi):
                    cb, no = ci // NO, ci % NO
                    if True:
                        npc, nof = no // (PW // 512), (no % (PW // 512)) * 512
                        psl = slice(nof, nof + 512)
                        # alternate acc through the bounce pool (idle
                        # during the A-phases): 4-deep epilogue pipeline
                        pool, tag = ((ps_o, "acc") if ci % 2 == 0
                                     else (ps_t, "bounce"))
                        acc = pool.tile([128, 512], F32, tag=tag,
                                        name=f"acc_{s}_{no}_{cb}")
                        for j in range(CB):
                            nc.tensor.matmul(
                                acc[:],
                                lhsT=expT[j][:, cb, :],
                                rhs=q16[j][npc][:, psl],
                                start=(j == 0),
                                stop=(j == CB - 1),
                            )
                        half = no // (NO // 2)
                        if (cb, half) not in ostage:
                            ot = outsp.tile([128, (NO // 2) * 512], F16,
                                            tag="ot", name=f"ot_{s}_{cb}_{half}")
                            ostage[(cb, half)] = ot
                        ot = ostage[(cb, half)]
                        osl = slice((no % (NO // 2)) * 512,
                                    (no % (NO // 2) + 1) * 512)
                        nc.vector.scalar_tensor_tensor(
                            out=ot[:, osl],
                            in0=acc[:],
                            scalar=scales[cb],
                            in1=q16[cb][npc][:, psl],
                            op0=mybir.AluOpType.mult,
                            op1=mybir.AluOpType.add,
                        )
                        rows = slice(s * C + cb * 128, s * C + (cb + 1) * 128)
                        base = half * (NO // 2) * 512
                        last = (s == 1 and cb == CB - 1 and half == 1)
                        if last:
                            hh = no % (NO // 2)
                            nc.sync.dma_start(
                                out=out_ap[rows, base + hh * 512 :
                                           base + (hh + 1) * 512],
                                in_=ot[:, hh * 512 : (hh + 1) * 512],
                            )
                        elif not last and no % (NO // 2) == NO // 2 - 1:
                            nc.sync.dma_start(
                                out=out_ap[rows, base : base + (NO // 2) * 512],
                                in_=ot[:],
                            )

            # ---- interleaved emission schedule -----------------------
            load(0)
            for g in range(NG):
                tgroup(0, g)
                emm(0, g)
            # sample-1 loads/casts/first transposes outrank softmax(0) so
            # DVE/ACT feed the PE through the softmax window; softmax(0)
            # has slack since A(0) runs only after E(1)
            load(1)
            for g in range(PRE):
                tgroup(1, g)
            softmax(0)
            expTf(0)
            for g in range(PRE):
                emm(1, g)
            for g in range(PRE, NG):
                tgroup(1, g)
                emm(1, g)
            # A(s0) emitted after E(s1); sample-1's softmax chain is
            # spread through a0's tail chunks so it hides completely:
            # copies/exp on ACT (idle during a0), rowmins rationed so
            # DVE's STT stream isn't starved
            aphase(0, 0, 20)
            softmax_esb_mirror(1)
            aphase(0, 20, 24)
            softmax_rowmin(1, [0, 1])
            aphase(0, 24, 26)
            softmax_rowmin(1, [2, 3])
            softmax_exp(1, [0, 1])
            aphase(0, 26, 28)
            softmax_exp(1, [2, 3])
            softmax_scales(1)
            aphase(0, 28, 32)
            expTf(1)
            aphase(1)
    return nc


def _split_excess_waits(nc, max_waits=1):
    """This container's walrus rejects >1 sync-wait on one instruction
    ("Too many sync wait commands"); hoist extras onto standalone
    InstEventSemaphore preludes on the same engine."""
    n = 0
    for fn in nc.m.functions:
        for bb in fn.blocks:
            out = []
            for inst in bb.instructions:
                si = inst.sync_info
                if si is not None and si.on_wait and len(si.on_wait) > max_waits:
                    waits = list(si.on_wait)
                    head, keep = waits[:-max_waits], waits[-max_waits:]
                    for i, w in enumerate(head):
                        ev = mybir.InstEventSemaphore(
                            name=f"{inst.name}-wsplit{i}", ins=[], outs=[])
                        ev.engine = inst.engine
                        ev.sync_info = mybir.SyncInfo(on_wait=[w], on_update=[])
                        out.append(ev)
                        n += 1
                    inst.sync_info = mybir.SyncInfo(
                        on_wait=keep, on_update=list(si.on_update))
                out.append(inst)
            bb.instructions[:] = out
    return n


_cache = {}


def _get_nc():
    if 'nc' not in _cache:
        nc = bass.Bass()
        build(nc)
        _split_excess_waits(nc)
        _cache['nc'] = nc
    return _cache['nc']


def kernel(x: np.ndarray, gamma: np.ndarray) -> np.ndarray:
    from concourse.bass_utils import run_bass_kernel_spmd

    B, CH, H, W = x.shape          # (16, 512, 64, 64)
    NSP = H * W
    M = 8                          # cores
    SS = B // M                    # samples per core
    nc = _get_nc()
    g = np.ascontiguousarray(gamma, dtype=np.float32).reshape(1, 1)
    in_maps = [
        {
            "x": np.ascontiguousarray(
                x[i * SS : (i + 1) * SS].reshape(SS * CH, NSP), dtype=np.float32
            ),
            "gamma": g,
        }
        for i in range(M)
    ]
    res = run_bass_kernel_spmd(nc, in_maps, core_ids=list(range(M)))
    out = np.concatenate(
        [res.results[i]["out"].astype(np.float32).reshape(SS, CH, H, W)
         for i in range(M)],
        axis=0,
    )
    return np.ascontiguousarray(out, dtype=np.float32)
